# revision 5
# baseline (speedup 1.0000x reference)
"""Trainium2 kernel for nn_BranchModel_9680856285960 (moe_routing).

Math: the reference's masked branch sum commutes with the feature
contraction, so the model collapses to a 3-layer dense MLP

    out = relu(relu(x @ Weff1.T) @ Weff2.T) @ W3 + b3

with Weff_l[o, i] = sum_{r,k} masks_l[ctx, r, o] * w_l[r, o, k]
* [idx_l[r, o, k] == i], folded host-side (free), plus dead-unit
pruning (2000 -> ~1792 alive units per layer, padded to 1792).

Sharding: PAIR-SPLIT hidden model parallelism. Cores (2p, 2p+1) co-own
batch rows [256p, 256p+256); each member computes its HALF of the
hidden units (896 = 7x128) through layers 1 and 2 over the full 256-row
group batch.  This halves the dominant per-core weight stream
(9.5 MB -> 5.3 MB, i.e. ~26 us -> ~15 us at ~360 GB/s) and makes the
kernel compute-bound (~16.5 us of fp16 matmul per core) as the
target_regime asks.  The unit halves are exchanged with ONE AllGather
per batch half over replica groups [[0,1],[2,3],[4,5],[6,7]] (DRAM to
DRAM, 224 KB each); layer-3 partial products [10, 256] are summed on
the host, so h2 is never exchanged.

Schedule highlights:
 - Single priority DMA queue (sync ring) streams x -> w1 chunks -> w2
   chunks -> w3 in consumption order at full ~360 GB/s; per-queue
   serialization gives w1 strict priority over w2 with zero plumbing.
 - Layer 1/2 run in two batch-half waves: the h1-half-0 AllGather and
   its SBUF readback overlap layer-1 compute of half 1, and layer 2 on
   half 0 overlaps the half-1 collective.
 - Layer-3 accumulation for chunk j is emitted after the layer-2 chain
   of chunk j+1, so the PE never stalls on the DVE relu latency.
 - No heartbeat junk: ~14 warm-up spins at kernel start lift the PE HAM
   clock gate (1.2 -> 2.4 GHz); afterwards the matmul stream is dense
   enough to keep it warm.  (The previous revision spent 18 us of PE
   time on filler matmuls that serialized with real work.)
"""

import os
import sys
import numpy as np

for _p in ("/opt/trn_rl_repo",):
    if os.path.isdir(_p) and _p not in sys.path:
        sys.path.append(_p)

from contextlib import ExitStack

from concourse import bass, mybir
import concourse.bacc as bacc
import concourse.tile as tile
from concourse.bass_utils import run_bass_kernel_spmd

F32 = mybir.dt.float32
F16 = mybir.dt.float16

BATCH, NIN, NH_FULL, NOUT = 1024, 784, 2000, 10
NCORES = 8
P = 128
BG = 256                      # group (pair) batch rows
HF = 128                      # batch half
GROUPS = [[0, 1], [2, 3], [4, 5], [6, 7]]


def _tiles(total, step):
    out, o = [], 0
    while o < total:
        out.append((o, min(step, total - o)))
        o += step
    return out


MT1 = _tiles(NIN, P)          # 7 contraction tiles over input features

LAST_RESULT = None
_CACHE = {}


def _build_weff(w, idx, mask_row, n_in):
    """Weff[o, i] = sum_{r,k} mask_row[r,o] * w[r,o,k] * [idx[r,o,k] == i]"""
    n_br, n_out, npb = w.shape
    acc = np.zeros(n_out * n_in, np.float64)
    base = (np.arange(n_out, dtype=np.int64) * n_in)[:, None]
    for r in range(n_br):
        flat = (base + idx[r].astype(np.int64)).ravel()
        vals = (w[r].astype(np.float64) * mask_row[r].astype(np.float64)[:, None]).ravel()
        acc += np.bincount(flat, weights=vals, minlength=n_out * n_in)
    return acc.reshape(n_out, n_in).astype(np.float32)


def _mlp_body(tc, nh, xT, w1p, w2p, w3p, h1x, h1g, out):
    nc = tc.nc
    nh2 = nh // 2
    JT = nh2 // P                 # unit chunks per member (7)
    CT2 = nh // P                 # layer-2 contraction tiles (14)
    nt1 = len(MT1)

    with ExitStack() as ctx:
        const = ctx.enter_context(tc.tile_pool(name="const", bufs=1))
        act = ctx.enter_context(tc.tile_pool(name="act", bufs=1))
        pacc = ctx.enter_context(tc.tile_pool(name="pacc", bufs=1, space="PSUM"))

        # Rotating PSUM banks shared by layer 1 and layer 2 (PSUM has only
        # 8 x 2KB banks; tiles are bank-granular).
        pp = [pacc.tile([P, HF], F32, name=f"pp_{i}", tag=f"pp_{i}")
              for i in range(4)]

        # PE warm-up against the HAM clock gate: garbage-in, discarded-out
        # (pp[0] is reset by the first real chain's start=True).
        wz = const.tile([P, BG], F16, tag="warmz")
        nc.vector.memset(wz[:, :1], 0.0)
        for _ in range(24):
            nc.tensor.matmul(pp[0][:], lhsT=wz[:, :P], rhs=wz[:, :HF],
                             start=True, stop=True)

        # Priority weight stream, one in-order queue: x, w1, w2, w3.
        xbig = const.tile([P, nt1, BG], F16, tag="xbig")
        nc.sync.dma_start(out=xbig[:], in_=xT)
        w1big = const.tile([P, JT, nt1, P], F16, tag="w1big")
        for j in range(JT):
            nc.sync.dma_start(out=w1big[:, j], in_=w1p[:, j])
        w2big = const.tile([P, JT, CT2, P], F16, tag="w2big")
        for j in range(JT):
            nc.sync.dma_start(out=w2big[:, j], in_=w2p[:, j])
        w3t = const.tile([P, JT, NOUT], F16, tag="w3")
        nc.sync.dma_start(out=w3t[:], in_=w3p)

        # ---- Layer 1 in two batch-half waves + per-half h1 AllGather.
        h1own = act.tile([P, 2, JT, HF], F16, tag="h1own")
        h1all = act.tile([P, 2, CT2, HF], F16, tag="h1all")
        for h in range(2):
            for j in range(JT):
                ps = pp[(h * JT + j) % 4]
                for t, (toff, tsz) in enumerate(MT1):
                    nc.tensor.matmul(
                        ps[:],
                        lhsT=w1big[:tsz, j, t, :],
                        rhs=xbig[:tsz, t, h * HF:(h + 1) * HF],
                        start=(t == 0),
                        stop=(t == nt1 - 1),
                    )
                nc.vector.tensor_scalar_max(h1own[:, h, j, :], ps[:], 0.0)
            # Exchange this batch half: stage to DRAM, AllGather with the
            # pair peer, read both members back (member order == unit order).
            nc.scalar.dma_start(out=h1x[h], in_=h1own[:, h])
            nc.gpsimd.collective_compute(
                "AllGather",
                mybir.AluOpType.bypass,
                replica_groups=GROUPS,
                ins=[h1x[h].opt()],
                outs=[h1g[h].opt()],
            )
            for m in range(2):
                nc.scalar.dma_start(
                    out=h1all[:, h, m * JT:(m + 1) * JT, :],
                    in_=h1g[h][m])

        # ---- Layer 2 + fused partial layer 3 (L3 deferred one chunk so
        # the PE never waits on the DVE relu).
        h2own = act.tile([P, 2, JT, HF], F16, tag="h2own")
        ps3 = [pacc.tile([NOUT, HF], F32, name=f"ps3_{h}", tag=f"ps3_{h}")
               for h in range(2)]

        def l3(h, j):
            nc.tensor.matmul(
                ps3[h][:],
                lhsT=w3t[:, j, :],
                rhs=h2own[:, h, j, :],
                start=(j == 0),
                stop=(j == JT - 1),
            )

        pending = None
        for h in range(2):
            for j in range(JT):
                ps = pp[(h * JT + j) % 4]
                for t in range(CT2):
                    nc.tensor.matmul(
                        ps[:],
                        lhsT=w2big[:, j, t, :],
                        rhs=h1all[:, h, t, :],
                        start=(t == 0),
                        stop=(t == CT2 - 1),
                    )
                nc.vector.tensor_scalar_max(h2own[:, h, j, :], ps[:], 0.0)
                if pending is not None:
                    l3(*pending)
                pending = (h, j)
        l3(*pending)

        o = act.tile([NOUT, BG], F32, tag="o")
        for h in range(2):
            nc.vector.tensor_scalar_add(o[:, h * HF:(h + 1) * HF], ps3[h][:], 0.0)
        nc.sync.dma_start(out=out, in_=o[:])


def _get_program(nh):
    key = ("pair", nh)
    if key in _CACHE:
        return _CACHE[key]
    nc = bacc.Bacc("TRN2", target_bir_lowering=False, debug=False,
                   enable_asserts=False, enable_partition_id=False,
                   num_devices=NCORES)
    nh2 = nh // 2
    JT = nh2 // P
    CT2 = nh // P
    nt1 = len(MT1)
    xT = nc.dram_tensor("xT", [P, nt1, BG], F16, kind="ExternalInput").ap()
    w1p = nc.dram_tensor("w1p", [P, JT, nt1, P], F16, kind="ExternalInput").ap()
    w2p = nc.dram_tensor("w2p", [P, JT, CT2, P], F16, kind="ExternalInput").ap()
    w3p = nc.dram_tensor("w3p", [P, JT, NOUT], F16, kind="ExternalInput").ap()
    h1x = [nc.dram_tensor(f"h1x{h}", [P, JT, HF], F16).ap() for h in range(2)]
    h1g = [nc.dram_tensor(f"h1g{h}", [2, P, JT, HF], F16).ap() for h in range(2)]
    out = nc.dram_tensor("out", [NOUT, BG], F32, kind="ExternalOutput").ap()
    with tile.TileContext(nc) as tc:
        _mlp_body(tc, nh, xT, w1p, w2p, w3p, h1x, h1g, out)
    nc.compile()
    _CACHE[key] = nc
    return nc


def kernel(x, w1, idx1, w2, idx2, masks1, masks2, W3, b3, context):
    global LAST_RESULT
    x = np.ascontiguousarray(np.asarray(x, dtype=np.float32))
    ctxi = int(np.asarray(context))
    m1 = np.asarray(masks1)[ctxi]
    m2 = np.asarray(masks2)[ctxi]

    # Dead-unit pruning; nh must be a multiple of 256 for the pair split.
    alive1 = np.where(m1.any(axis=0))[0]
    alive2 = np.where(m2.any(axis=0))[0]
    nh = max(len(alive1), len(alive2))
    nh = max(2 * P, -(-nh // (2 * P)) * (2 * P))
    nh2 = nh // 2
    JT = nh2 // P
    CT2 = nh // P
    nt1 = len(MT1)

    weff1 = _build_weff(np.asarray(w1), np.asarray(idx1), m1, NIN)
    weff2 = _build_weff(np.asarray(w2), np.asarray(idx2), m2, NH_FULL)

    w1t = np.zeros((NIN, nh), np.float16)          # [feat, alive1-unit]
    w1t[:, :len(alive1)] = weff1[alive1, :].T.astype(np.float16)
    w2t = np.zeros((nh, nh), np.float16)           # [alive1-unit, alive2-unit]
    w2t[:len(alive1), :len(alive2)] = \
        weff2[np.ix_(alive2, alive1)].T.astype(np.float16)
    w3f = np.zeros((nh, NOUT), np.float16)
    w3f[:len(alive2)] = np.asarray(W3)[alive2, :].astype(np.float16)

    try:
        import antenv.axon_hooks  # noqa: F401
    except Exception:
        os.environ.setdefault("BASS_NEVER_TRACE", "1")

    nc = _get_program(nh)

    # Member-sliced weight packs (member m of a pair owns unit columns
    # [m*nh2, (m+1)*nh2) of both layers).
    w1mp, w2mp, w3mp = [], [], []
    for m in range(2):
        w1pk = np.zeros((P, JT, nt1, P), np.float16)
        w2pk = np.zeros((P, JT, CT2, P), np.float16)
        w3pk = np.zeros((P, JT, NOUT), np.float16)
        for j in range(JT):
            u0 = m * nh2 + j * P
            for t, (toff, tsz) in enumerate(MT1):
                w1pk[:tsz, j, t, :] = w1t[toff:toff + tsz, u0:u0 + P]
            for t in range(CT2):
                w2pk[:, j, t, :] = w2t[t * P:(t + 1) * P, u0:u0 + P]
            w3pk[:, j, :] = w3f[u0:u0 + P, :]
        w1mp.append(w1pk)
        w2mp.append(w2pk)
        w3mp.append(w3pk)

    in_maps = []
    for c in range(NCORES):
        pair, m = c // 2, c % 2
        xs = x[pair * BG:(pair + 1) * BG].T.astype(np.float16)  # [784, 256]
        xTp = np.zeros((P, nt1, BG), np.float16)
        for t, (toff, tsz) in enumerate(MT1):
            xTp[:tsz, t, :] = xs[toff:toff + tsz, :]
        in_maps.append({"xT": xTp, "w1p": w1mp[m], "w2p": w2mp[m],
                        "w3p": w3mp[m]})

    LAST_RESULT = run_bass_kernel_spmd(nc, in_maps, list(range(NCORES)))

    b3f = np.asarray(b3, dtype=np.float32)
    outs = []
    for pair in range(NCORES // 2):
        o = (LAST_RESULT.results[2 * pair]["out"].astype(np.float32)
             + LAST_RESULT.results[2 * pair + 1]["out"].astype(np.float32))
        outs.append(o.T + b3f)                     # [256, 10]
    return np.concatenate(outs, axis=0)


# revision 6
# speedup vs baseline: 1.7917x; 1.7917x over previous
"""Trainium2 kernel for nn_BranchModel_9680856285960 (moe_routing).

Math: the reference's masked branch sum commutes with the feature
contraction, so the model collapses to a 3-layer dense MLP

    out = relu(relu(x @ Weff1.T) @ Weff2.T) @ W3 + b3

with Weff_l[o, i] = sum_{r,k} masks_l[ctx, r, o] * w_l[r, o, k]
* [idx_l[r, o, k] == i], folded host-side (free), plus dead-unit
pruning (2000 -> ~1792 alive units per layer, padded to 1792).

Sharding: PAIR-SPLIT hidden model parallelism.  Cores (2p, 2p+1) co-own
batch rows [256p, 256p+256); each member computes its HALF of the
hidden units (896 = 7x128) through layers 1 and 2 over the full 256-row
group batch.  This halves the dominant per-core weight stream
(9.5 MB -> ~5 MB) and makes the kernel compute-bound (~16.5 us of fp16
matmul per core).  Layer-3 partial products [10, 256] are summed on the
host, so h2 is never exchanged.

The h1 halves are swapped SBUF-to-SBUF with remote_dma_broadcast using
RELATIVE dests (delta_tpb = 1): the hardware XORs the destination with
the core's own tpb index, so one uniform SPMD program swaps every pair.
Each of the 7 h1 chunks rides its own broadcast SLOT (distinct DMA-lane
pair), so the seven 64KB transfers run in parallel right behind the
producing relu.  SBUF h1 is laid out own-half-first on every core; the
per-core host packing of the w2 row blocks compensates, so no
per-core instructions are needed anywhere.

(DRAM-collective AllGather was tried first and measured ~25 us per
224KB pair-gather end to end -- the CC path has enormous fixed cost on
this part -- hence the raw remote-DMA swap.  PAIR_EXCHANGE=0 falls back
to computing both halves of h1 locally from a replicated w1.)

Schedule highlights:
 - Two round-robin DMA queues (sync+scalar rings) stream x -> w1 chunks
   -> w2 chunks -> w3 in consumption order; per-queue ordering keeps w1
   strictly ahead of w2.
 - Layer-3 accumulation for chunk j is emitted after the layer-2 chain
   of chunk j+1, so the PE never stalls on the DVE relu latency.
 - No heartbeat junk: ~24 warm-up spins at kernel start lift the PE HAM
   clock gate (1.2 -> 2.4 GHz); afterwards the matmul stream is dense
   enough to keep it warm.
"""

import os
import sys
import numpy as np

for _p in ("/opt/trn_rl_repo",):
    if os.path.isdir(_p) and _p not in sys.path:
        sys.path.append(_p)

from contextlib import ExitStack

from concourse import bass, mybir
import concourse.bacc as bacc
import concourse.tile as tile
from concourse.bass_utils import run_bass_kernel_spmd

F32 = mybir.dt.float32
F16 = mybir.dt.float16

BATCH, NIN, NH_FULL, NOUT = 1024, 784, 2000, 10
NCORES = 8
P = 128
BG = 256                      # group (pair) batch rows

PAIR_EXCHANGE = os.environ.get("KERNEL_PAIR_EXCHANGE", "1") == "1"


def _tiles(total, step):
    out, o = [], 0
    while o < total:
        out.append((o, min(step, total - o)))
        o += step
    return out


MT1 = _tiles(NIN, P)          # 7 contraction tiles over input features

LAST_RESULT = None
_CACHE = {}


def _build_weff(w, idx, mask_row, n_in):
    """Weff[o, i] = sum_{r,k} mask_row[r,o] * w[r,o,k] * [idx[r,o,k] == i]"""
    n_br, n_out, npb = w.shape
    acc = np.zeros(n_out * n_in, np.float64)
    base = (np.arange(n_out, dtype=np.int64) * n_in)[:, None]
    for r in range(n_br):
        flat = (base + idx[r].astype(np.int64)).ravel()
        vals = (w[r].astype(np.float64) * mask_row[r].astype(np.float64)[:, None]).ravel()
        acc += np.bincount(flat, weights=vals, minlength=n_out * n_in)
    return acc.reshape(n_out, n_in).astype(np.float32)


def _mlp_body(tc, nh, xT, w1p, w2p, w3p, out, exchange):
    nc = tc.nc
    nh2 = nh // 2
    JT = nh2 // P                 # unit chunks per member (7)
    CT2 = nh // P                 # layer-2 contraction tiles (14)
    L1J = JT if exchange else CT2  # h1 chunks computed locally
    nt1 = len(MT1)

    rsem = lsem = None
    if exchange:
        rsem = nc.alloc_semaphore("pair_rsem")
        lsem = nc.alloc_semaphore("pair_lsem")

    with ExitStack() as ctx:
        const = ctx.enter_context(tc.tile_pool(name="const", bufs=1))
        act = ctx.enter_context(tc.tile_pool(name="act", bufs=1))
        pacc = ctx.enter_context(tc.tile_pool(name="pacc", bufs=1, space="PSUM"))

        # Rotating PSUM banks shared by layer 1 and layer 2 (PSUM has only
        # 8 x 2KB banks; tiles are bank-granular).
        pp = [pacc.tile([P, BG], F32, name=f"pp_{i}", tag=f"pp_{i}")
              for i in range(4)]

        # PE warm-up against the HAM clock gate: garbage-in, discarded-out
        # (pp[0] is reset by the first real chain's start=True).
        wz = const.tile([P, BG], F16, tag="warmz")
        nc.vector.memset(wz[:, :1], 0.0)
        for _ in range(14):
            nc.tensor.matmul(pp[0][:], lhsT=wz[:, :P], rhs=wz[:],
                             start=True, stop=True)

        # Priority weight stream on two round-robin in-order queues.
        qs = [nc.sync, nc.scalar]
        qi = 0

        def stream(dst, src):
            nonlocal qi
            qs[qi % 2].dma_start(out=dst, in_=src)
            qi += 1

        xbig = const.tile([P, nt1, BG], F16, tag="xbig")
        stream(xbig[:], xT)
        w1big = const.tile([P, L1J, nt1, P], F16, tag="w1big")
        for j in range(L1J):
            stream(w1big[:, j], w1p[:, j])
        w2big = const.tile([P, JT, CT2, P], F16, tag="w2big")
        for j in range(JT):
            stream(w2big[:, j], w2p[:, j])
        w3t = const.tile([P, JT, NOUT], F16, tag="w3")
        stream(w3t[:], w3p)

        # ---- Layer 1 (+ pairwise h1 half-swap in exchange mode).
        h1all = act.tile([P, CT2, BG], F16, tag="h1all")
        if exchange:
            # Touch the remote-written half so the tile exists for Tile's
            # book-keeping before the peer's broadcast lands.
            nc.vector.memset(h1all[:, JT:, :1], 0.0)
        for j in range(L1J):
            ps = pp[j % 4]
            for t, (toff, tsz) in enumerate(MT1):
                nc.tensor.matmul(
                    ps[:],
                    lhsT=w1big[:tsz, j, t, :],
                    rhs=xbig[:tsz, t, :],
                    start=(t == 0),
                    stop=(t == nt1 - 1),
                )
            nc.vector.tensor_scalar_max(h1all[:, j, :], ps[:], 0.0)
            if exchange:
                # Ship chunk j to the pair peer's h1all peer-half, slot j
                # (XOR-relative dest: tpb ^= 1). 2 DMA lanes per slot; the
                # 7 chunks ride disjoint lanes and run concurrently.
                nc.gpsimd.remote_dma_broadcast(
                    out_ap=h1all[:, JT + j, :],
                    in_ap=h1all[:, j, :],
                    remote_sem=rsem,
                    local_sem=lsem,
                    rdests=[(0, 1) if k == j else None for k in range(8)],
                )
                nc.gpsimd.trigger_dma(count=None)

        if exchange:
            # All 7 peer chunks landed (2 rsem increments per broadcast).
            nc.tensor.wait_ge(rsem, 2 * JT)

        # ---- Layer 2 + fused partial layer 3 (L3 deferred one chunk so
        # the PE never waits on the DVE relu).
        h2own = act.tile([P, JT, BG], F16, tag="h2own")
        ps3 = pacc.tile([NOUT, BG], F32, tag="ps3")

        def l3(j):
            nc.tensor.matmul(
                ps3[:],
                lhsT=w3t[:, j, :],
                rhs=h2own[:, j, :],
                start=(j == 0),
                stop=(j == JT - 1),
            )

        pending = None
        for j in range(JT):
            ps = pp[j % 4]
            for t in range(CT2):
                nc.tensor.matmul(
                    ps[:],
                    lhsT=w2big[:, j, t, :],
                    rhs=h1all[:, t, :],
                    start=(t == 0),
                    stop=(t == CT2 - 1),
                )
            nc.vector.tensor_scalar_max(h2own[:, j, :], ps[:], 0.0)
            if pending is not None:
                l3(pending)
            pending = j
        l3(pending)

        o = act.tile([NOUT, BG], F32, tag="o")
        nc.vector.tensor_scalar_add(o[:], ps3[:], 0.0)
        nc.sync.dma_start(out=out, in_=o[:])

        if exchange:
            # Our own sends must have left SBUF before the kernel retires.
            nc.gpsimd.wait_ge(lsem, 16 * JT)


def _get_program(nh, exchange):
    key = ("pair", nh, exchange)
    if key in _CACHE:
        return _CACHE[key]
    nc = bacc.Bacc("TRN2", target_bir_lowering=False, debug=False,
                   enable_asserts=False, enable_partition_id=False,
                   num_devices=NCORES)
    nh2 = nh // 2
    JT = nh2 // P
    CT2 = nh // P
    L1J = JT if exchange else CT2
    nt1 = len(MT1)
    xT = nc.dram_tensor("xT", [P, nt1, BG], F16, kind="ExternalInput").ap()
    w1p = nc.dram_tensor("w1p", [P, L1J, nt1, P], F16, kind="ExternalInput").ap()
    w2p = nc.dram_tensor("w2p", [P, JT, CT2, P], F16, kind="ExternalInput").ap()
    w3p = nc.dram_tensor("w3p", [P, JT, NOUT], F16, kind="ExternalInput").ap()
    out = nc.dram_tensor("out", [NOUT, BG], F32, kind="ExternalOutput").ap()
    with tile.TileContext(nc) as tc:
        _mlp_body(tc, nh, xT, w1p, w2p, w3p, out, exchange)
    nc.compile()
    _CACHE[key] = nc
    return nc


def kernel(x, w1, idx1, w2, idx2, masks1, masks2, W3, b3, context):
    global LAST_RESULT
    exchange = PAIR_EXCHANGE
    x = np.ascontiguousarray(np.asarray(x, dtype=np.float32))
    ctxi = int(np.asarray(context))
    m1 = np.asarray(masks1)[ctxi]
    m2 = np.asarray(masks2)[ctxi]

    # Dead-unit pruning; nh must be a multiple of 256 for the pair split.
    alive1 = np.where(m1.any(axis=0))[0]
    alive2 = np.where(m2.any(axis=0))[0]
    nh = max(len(alive1), len(alive2))
    nh = max(2 * P, -(-nh // (2 * P)) * (2 * P))
    nh2 = nh // 2
    JT = nh2 // P
    CT2 = nh // P
    L1J = JT if exchange else CT2
    nt1 = len(MT1)

    weff1 = _build_weff(np.asarray(w1), np.asarray(idx1), m1, NIN)
    weff2 = _build_weff(np.asarray(w2), np.asarray(idx2), m2, NH_FULL)

    w1t = np.zeros((NIN, nh), np.float16)          # [feat, alive1-unit]
    w1t[:, :len(alive1)] = weff1[alive1, :].T.astype(np.float16)
    w2t = np.zeros((nh, nh), np.float16)           # [alive1-unit, alive2-unit]
    w2t[:len(alive1), :len(alive2)] = \
        weff2[np.ix_(alive2, alive1)].T.astype(np.float16)
    w3f = np.zeros((nh, NOUT), np.float16)
    w3f[:len(alive2)] = np.asarray(W3)[alive2, :].astype(np.float16)

    try:
        import antenv.axon_hooks  # noqa: F401
    except Exception:
        os.environ.setdefault("BASS_NEVER_TRACE", "1")

    nc = _get_program(nh, exchange)

    # Member-sliced packs.  SBUF h1 tiles are own-half-first, so member
    # m's layer-1 pack covers its own unit columns and its layer-2 row
    # blocks are rotated to match (row tile t holds global row block
    # (t + m*JT) % CT2).  Without exchange, layer 1 is computed in full
    # on every core and all layouts are in natural global order.
    w1mp, w2mp, w3mp = [], [], []
    for m in range(2):
        w1pk = np.zeros((P, L1J, nt1, P), np.float16)
        w2pk = np.zeros((P, JT, CT2, P), np.float16)
        w3pk = np.zeros((P, JT, NOUT), np.float16)
        for j in range(L1J):
            u0 = (m * nh2 if exchange else 0) + j * P
            for t, (toff, tsz) in enumerate(MT1):
                w1pk[:tsz, j, t, :] = w1t[toff:toff + tsz, u0:u0 + P]
        for j in range(JT):
            u0 = m * nh2 + j * P
            for t in range(CT2):
                rt = (t + m * JT) % CT2 if exchange else t
                w2pk[:, j, t, :] = w2t[rt * P:(rt + 1) * P, u0:u0 + P]
            w3pk[:, j, :] = w3f[u0:u0 + P, :]
        w1mp.append(w1pk)
        w2mp.append(w2pk)
        w3mp.append(w3pk)

    in_maps = []
    for c in range(NCORES):
        pair, m = c // 2, c % 2
        xs = x[pair * BG:(pair + 1) * BG].T.astype(np.float16)  # [784, 256]
        xTp = np.zeros((P, nt1, BG), np.float16)
        for t, (toff, tsz) in enumerate(MT1):
            xTp[:tsz, t, :] = xs[toff:toff + tsz, :]
        in_maps.append({"xT": xTp, "w1p": w1mp[m], "w2p": w2mp[m],
                        "w3p": w3mp[m]})

    LAST_RESULT = run_bass_kernel_spmd(nc, in_maps, list(range(NCORES)))

    b3f = np.asarray(b3, dtype=np.float32)
    outs = []
    for pair in range(NCORES // 2):
        o = (LAST_RESULT.results[2 * pair]["out"].astype(np.float32)
             + LAST_RESULT.results[2 * pair + 1]["out"].astype(np.float32))
        outs.append(o.T + b3f)                     # [256, 10]
    return np.concatenate(outs, axis=0)
